# revision 21
# baseline (speedup 1.0000x reference)
"""Causal MHA (B=4, T=2048, D=1024, H=16) on 8 trn2 cores.

Sharding: core c = (batch b = c//2, head-group g = c%2). Each core computes
QKV projections for its 8 heads, causal attention, and the row-parallel
out-proj partial product. Host sums the two partials per batch + bias.

On-device layout (per core):
  xT   [1024, 2048]  X^T (d on partitions)           bf16
  QT/KT [512, 2048]  Q^T/K^T (e=head*64+d rows)      bf16
  V_pad [2048, 520]  V natural + ones col per head   bf16
  scores S^T tiles [128 k, 2x512 q] (2 heads/psum), exp on ScalarE,
  ctx = P^T-stationary matmul -> [128 q, 65] (col 64 = softmax denom),
  normalize per-partition, PE-transpose -> ctx^T, out-proj partial.
"""

import os

import numpy as np
import ml_dtypes

import concourse.bass as bass
import concourse.bacc as bacc
import concourse.tile as tile
from concourse import mybir
from concourse.bass_utils import run_bass_kernel_spmd
from concourse.masks import make_identity

BF16 = ml_dtypes.bfloat16

B, T, D = 4, 2048, 1024
H, HD = 16, 64
E = 512          # per-core projection width (8 heads * 64)
DC = D // 128    # 8 contraction chunks
EC = E // 128    # 4 e chunks (head pairs)
TJ = T // 512    # 4 q-chunks of 512
TQ = T // 128    # 16 t-chunks of 128

F32 = mybir.dt.float32
BF = mybir.dt.bfloat16

LAST = {}
_CACHE = {}


def _build():
    nc = bacc.Bacc("TRN2")
    xT = nc.dram_tensor("xT", [D, T], BF, kind="ExternalInput")
    wq = nc.dram_tensor("wq", [D, E], BF, kind="ExternalInput")
    wk = nc.dram_tensor("wk", [D, E], BF, kind="ExternalInput")
    wv = nc.dram_tensor("wv", [D, E], BF, kind="ExternalInput")
    wo = nc.dram_tensor("wo", [E, D], BF, kind="ExternalInput")
    msk = nc.dram_tensor("msk", [4, 128, 512], BF, kind="ExternalInput")
    outp = nc.dram_tensor("out", [T, D], F32, kind="ExternalOutput")

    with tile.TileContext(nc) as tc:
        with (
            tc.tile_pool(name="const", bufs=1) as const,
            tc.tile_pool(name="acts", bufs=1) as acts,
            tc.tile_pool(name="ppool", bufs=22) as ppool,
            tc.tile_pool(name="small", bufs=6) as small,
            tc.tile_pool(name="stage", bufs=6) as stage,
            tc.tile_pool(name="obuf", bufs=2) as obufp,
            tc.tile_pool(name="psS", bufs=3, space="PSUM") as psS,
            tc.tile_pool(name="psP", bufs=2, space="PSUM") as psP,
            tc.tile_pool(name="psC", bufs=2, space="PSUM") as psC,
            tc.tile_pool(name="psT", bufs=1, space="PSUM") as psT,
        ):
            # ---------- constants; DMA order follows the critical path:
            # wq/wk + xT gate the first projections, wv/wo/msk can trail
            wq_sb = const.tile([128, DC, E], BF, tag="wq")
            wk_sb = const.tile([128, DC, E], BF, tag="wk")
            wv_sb = const.tile([128, DC, E], BF, tag="wv")
            wo_sb = const.tile([128, EC, D], BF, tag="wo")
            msk_sb = const.tile([128, 4, 512], BF, tag="msk")
            ident = const.tile([128, 128], BF, tag="ident")
            xT_sb = acts.tile([128, DC, T], BF, tag="xT")

            nc.sync.dma_start(out=wq_sb, in_=wq.rearrange("(dc p) e -> p dc e", p=128))
            nc.sync.dma_start(out=wk_sb, in_=wk.rearrange("(dc p) e -> p dc e", p=128))
            for dc in range(DC):
                nc.sync.dma_start(
                    out=xT_sb[:, dc, :], in_=xT[dc * 128 : (dc + 1) * 128, :]
                )
            nc.sync.dma_start(
                out=wv_sb, in_=wv.rearrange("(dc p) e -> p dc e", p=128)
            )
            nc.sync.dma_start(
                out=msk_sb, in_=msk.rearrange("r p f -> p r f")
            )
            nc.sync.dma_start(
                out=wo_sb, in_=wo.rearrange("(ec p) o -> p ec o", p=128)
            )
            make_identity(nc, ident)

            QT_sb = acts.tile([128, EC, T], BF, tag="QT")
            KT_sb = acts.tile([128, EC, T], BF, tag="KT")
            V_sb = acts.tile([128, TQ, 8 * 65], BF, tag="V")
            CT_sb = acts.tile([128, EC, T], BF, tag="CT")

            # ones columns only (col 64 of each per-head 65-group) so the V
            # copies below touch disjoint bytes and carry no WAW dep here
            for t7 in range(TQ):
                nc.vector.memset(
                    V_sb[:, t7, :].rearrange("p (h d) -> p h d", d=65)[:, :, 64:65],
                    1.0,
                )

            # ---------- op factories (emitted interleaved, see queue below)
            def qk_proj_ops(cp):
                ops = []
                for dst, w_sb in ((QT_sb, wq_sb), (KT_sb, wk_sb)):
                    for t5 in range(TJ):

                        def op(dst=dst, w_sb=w_sb, cp=cp, t5=t5):
                            ps = psP.tile([128, 512], F32, tag="psP")
                            for dc in range(DC):
                                nc.tensor.matmul(
                                    ps,
                                    lhsT=w_sb[:, dc, cp * 128 : (cp + 1) * 128],
                                    rhs=xT_sb[:, dc, t5 * 512 : (t5 + 1) * 512],
                                    start=(dc == 0),
                                    stop=(dc == DC - 1),
                                )
                            nc.vector.tensor_copy(
                                out=dst[:, cp, t5 * 512 : (t5 + 1) * 512], in_=ps
                            )

                        ops.append(op)
                return ops

            def v_ops():
                ops = []
                for t7 in range(TQ):

                    def op(t7=t7):
                        ps = psP.tile([128, 512], F32, tag="psP")
                        for dc in range(DC):
                            nc.tensor.matmul(
                                ps,
                                lhsT=xT_sb[:, dc, t7 * 128 : (t7 + 1) * 128],
                                rhs=wv_sb[:, dc, :],
                                start=(dc == 0),
                                stop=(dc == DC - 1),
                            )
                        for h8 in range(8):
                            nc.vector.tensor_copy(
                                out=V_sb[:, t7, h8 * 65 : h8 * 65 + 64],
                                in_=ps[:, h8 * 64 : (h8 + 1) * 64],
                            )

                    ops.append(op)
                return ops

            def outproj_ops(j):
                ops = []
                for t7 in range(4 * j, 4 * j + 4):

                    def op(t7=t7):
                        ob = obufp.tile([128, 1024], F32, tag="obuf")
                        for oc in range(2):
                            ps = psP.tile([128, 512], F32, tag="psP")
                            for ec in range(EC):
                                nc.tensor.matmul(
                                    ps,
                                    lhsT=CT_sb[:, ec, t7 * 128 : (t7 + 1) * 128],
                                    rhs=wo_sb[:, ec, oc * 512 : (oc + 1) * 512],
                                    start=(ec == 0),
                                    stop=(ec == EC - 1),
                                )
                            nc.scalar.copy(
                                out=ob[:, oc * 512 : (oc + 1) * 512], in_=ps
                            )
                        nc.sync.dma_start(
                            out=outp[t7 * 128 : (t7 + 1) * 128, :], in_=ob
                        )

                    ops.append(op)
                return ops

            def ctx_ops(cp, j, pts):
                ops = []
                for qr in range(4):
                    qc = 4 * j + qr
                    cn = stage.tile([128, 128], BF, tag="ctxn")
                    for h in range(2):

                        def mm_group(cp=cp, h=h, qr=qr, qc=qc, pts=pts, cn=cn):
                            habs = 2 * cp + h
                            cps = psC.tile([128, 65], F32, tag="psC")
                            for i in range(qc + 1):
                                nc.tensor.matmul(
                                    cps,
                                    lhsT=pts[i][:, h, qr * 128 : (qr + 1) * 128],
                                    rhs=V_sb[:, i, habs * 65 : habs * 65 + 65],
                                    start=(i == 0),
                                    stop=(i == qc),
                                )
                            rc = small.tile([128, 1], F32, tag="recip")
                            nc.vector.reciprocal(rc, cps[:, 64:65])
                            nc.vector.tensor_scalar_mul(
                                out=cn[:, 64 * h : 64 * h + 64],
                                in0=cps[:, 0:64],
                                scalar1=rc,
                            )

                        ops.append(mm_group)

                    def finish(cp=cp, qc=qc, cn=cn):
                        tps = psT.tile([128, 128], BF, tag="psT")
                        nc.tensor.transpose(tps, cn, ident)
                        nc.vector.tensor_copy(
                            out=CT_sb[:, cp, qc * 128 : (qc + 1) * 128], in_=tps
                        )

                    ops.append(finish)
                return ops

            # ---------- pipelined emission ----------
            # S/exp/mask stream is ACT-bound; all other PE work (projections,
            # V, ctx of the previous iteration, out-proj) drains through the
            # two queues between S steps so the PE never starves.
            for op in qk_proj_ops(0):
                op()
            slow = list(v_ops())
            fast = []
            for cp in range(EC):
                if cp < EC - 1:
                    slow.extend(qk_proj_ops(cp + 1))
                steps_left = 40
                for j in range(TJ):
                    nk = 4 * j + 4
                    ds = max(1, nk // 2)
                    fper = (len(fast) + ds - 1) // ds
                    pts = []
                    for i in range(nk):
                        pt = ppool.tile([128, 2, 512], BF, tag="P")
                        for h in range(2):
                            lo = 64 * h
                            sh = psS.tile([128, 512], F32, tag="psS")
                            nc.tensor.matmul(
                                sh,
                                lhsT=KT_sb[lo : lo + 64, cp, i * 128 : (i + 1) * 128],
                                rhs=QT_sb[lo : lo + 64, cp, j * 512 : (j + 1) * 512],
                                start=True,
                                stop=True,
                            )
                            nc.scalar.activation(
                                out=pt[:, h, :],
                                in_=sh,
                                func=mybir.ActivationFunctionType.Exp,
                                scale=0.125,
                            )
                        r = i - 4 * j
                        if r >= 0:  # diagonal tile: causal mask (f >= p+128r)
                            for h in range(2):
                                nc.vector.tensor_mul(
                                    pt[:, h, :], pt[:, h, :], msk_sb[:, r, :]
                                )
                        pts.append(pt)
                        for _ in range(fper):
                            if fast:
                                fast.pop(0)()
                        spr = (len(slow) + steps_left - 1) // steps_left
                        for _ in range(spr):
                            if slow:
                                slow.pop(0)()
                        steps_left -= 1
                    while fast:
                        fast.pop(0)()
                    fast = ctx_ops(cp, j, pts)
                    if cp == EC - 1:
                        # out-proj for these q rows AFTER their ctx writes in
                        # the same FIFO: queued ops can only depend on already
                        # emitted producers
                        fast.extend(outproj_ops(j))
            while fast:
                fast.pop(0)()
            while slow:
                slow.pop(0)()
    nc.compile()
    return nc


def _get_nc():
    if "nc" not in _CACHE:
        _CACHE["nc"] = _build()
    return _CACHE["nc"]


def _ensure_ntff_hook():
    """Install the axon NTFF profiling hook if the image's antenv lacks it."""
    import sys
    import types

    try:
        import antenv.axon_hooks  # noqa: F401

        return
    except ImportError:
        pass
    try:
        import antenv

        mod = types.ModuleType("antenv.axon_hooks")
        holder = {"hook": None}
        mod.set_axon_ntff_profile_hook = lambda h: holder.__setitem__("hook", h)
        mod.get_axon_ntff_profile_hook = lambda: holder["hook"]
        sys.modules["antenv.axon_hooks"] = mod
        antenv.axon_hooks = mod
        from trn_agent_boot.trn_boot import _ntff_profile_via_ctypes

        so = "/opt/axon/libaxon_pjrt.so"
        if os.path.exists(so):
            mod.set_axon_ntff_profile_hook(_ntff_profile_via_ctypes(so))
    except Exception:
        pass


def _masks():
    p = np.arange(128)[:, None]
    f = np.arange(512)[None, :]
    return np.stack(
        [(f >= p + 128 * r).astype(np.float32) for r in range(4)]
    ).astype(BF16)


def kernel(inputs, Wq, Wk, Wv, Wo, bo):
    inputs = np.asarray(inputs, dtype=np.float32)
    Wq = np.asarray(Wq, dtype=np.float32)
    Wk = np.asarray(Wk, dtype=np.float32)
    Wv = np.asarray(Wv, dtype=np.float32)
    Wo = np.asarray(Wo, dtype=np.float32)
    bo = np.asarray(bo, dtype=np.float32)

    nc = _get_nc()
    msk = _masks()
    wqs = [np.ascontiguousarray(Wq[:, g * E : (g + 1) * E]).astype(BF16) for g in range(2)]
    wks = [np.ascontiguousarray(Wk[:, g * E : (g + 1) * E]).astype(BF16) for g in range(2)]
    wvs = [np.ascontiguousarray(Wv[:, g * E : (g + 1) * E]).astype(BF16) for g in range(2)]
    wos = [np.ascontiguousarray(Wo[g * E : (g + 1) * E, :]).astype(BF16) for g in range(2)]
    xTs = [np.ascontiguousarray(inputs[b].T).astype(BF16) for b in range(B)]

    in_maps = []
    for c in range(8):
        b, g = divmod(c, 2)
        in_maps.append(
            {
                "xT": xTs[b],
                "wq": wqs[g],
                "wk": wks[g],
                "wv": wvs[g],
                "wo": wos[g],
                "msk": msk,
            }
        )

    trace = os.environ.get("KERNEL_TRACE", "0") == "1"
    if trace:
        _ensure_ntff_hook()
    tcores = None
    if os.environ.get("KERNEL_TRACE_ALL", "0") == "1":
        tcores = list(range(8))
    res = run_bass_kernel_spmd(
        nc, in_maps, core_ids=list(range(8)), trace=trace, trace_cores=tcores
    )
    LAST["exec_ns"] = res.exec_time_ns
    LAST["trace"] = res.instructions_and_trace
    LAST["profile_json"] = res.profile_json

    out = np.empty((B, T, D), np.float32)
    for b in range(B):
        out[b] = res.results[2 * b]["out"] + res.results[2 * b + 1]["out"] + bo[None, :]
    return out


# revision 22
# speedup vs baseline: 1.1957x; 1.1957x over previous
"""Causal MHA (B=4, T=2048, D=1024, H=16) on 8 trn2 cores.

Sharding: core c = (batch b = c//2, head-group g = c%2). Each core computes
QKV projections for its 8 heads, causal attention, and the row-parallel
out-proj partial product. Host sums the two partials per batch + bias.

On-device layout (per core):
  xT   [1024, 2048]  X^T (d on partitions)           bf16
  QT/KT [512, 2048]  Q^T/K^T (e=head*64+d rows)      bf16
  V_pad [2048, 520]  V natural + ones col per head   bf16
  scores S^T tiles [128 k, 2x512 q] (2 heads/psum), exp on ScalarE,
  ctx = P^T-stationary matmul -> [128 q, 65] (col 64 = softmax denom),
  normalize per-partition, PE-transpose -> ctx^T, out-proj partial.
"""

import os

import numpy as np
import ml_dtypes

import concourse.bass as bass
import concourse.bacc as bacc
import concourse.tile as tile
from concourse import mybir
from concourse.bass_utils import run_bass_kernel_spmd
from concourse.masks import make_identity

BF16 = ml_dtypes.bfloat16

B, T, D = 4, 2048, 1024
H, HD = 16, 64
E = 512          # per-core projection width (8 heads * 64)
DC = D // 128    # 8 contraction chunks
EC = E // 128    # 4 e chunks (head pairs)
TJ = T // 512    # 4 q-chunks of 512
TQ = T // 128    # 16 t-chunks of 128

F32 = mybir.dt.float32
BF = mybir.dt.bfloat16

LAST = {}
_CACHE = {}


def _build():
    nc = bacc.Bacc("TRN2")
    xT = nc.dram_tensor("xT", [D, T], BF, kind="ExternalInput")
    wq = nc.dram_tensor("wq", [D, E], BF, kind="ExternalInput")
    wk = nc.dram_tensor("wk", [D, E], BF, kind="ExternalInput")
    wv = nc.dram_tensor("wv", [D, E], BF, kind="ExternalInput")
    wo = nc.dram_tensor("wo", [E, D], BF, kind="ExternalInput")
    msk = nc.dram_tensor("msk", [4, 128, 512], BF, kind="ExternalInput")
    outp = nc.dram_tensor("out", [T, D], F32, kind="ExternalOutput")

    with tile.TileContext(nc) as tc:
        with (
            tc.tile_pool(name="const", bufs=1) as const,
            tc.tile_pool(name="acts", bufs=1) as acts,
            tc.tile_pool(name="ppool", bufs=22) as ppool,
            tc.tile_pool(name="small", bufs=6) as small,
            tc.tile_pool(name="stage", bufs=6) as stage,
            tc.tile_pool(name="obuf", bufs=2) as obufp,
            tc.tile_pool(name="psS", bufs=2, space="PSUM") as psS,
            tc.tile_pool(name="psP", bufs=2, space="PSUM") as psP,
            tc.tile_pool(name="psC", bufs=2, space="PSUM") as psC,
        ):
            # ---------- constants; DMA order follows the critical path:
            # wq/wk + xT gate the first projections, wv/wo/msk can trail
            wq_sb = const.tile([128, DC, E], BF, tag="wq")
            wk_sb = const.tile([128, DC, E], BF, tag="wk")
            wv_sb = const.tile([128, DC, E], BF, tag="wv")
            wo_sb = const.tile([128, EC, D], BF, tag="wo")
            msk_sb = const.tile([128, 4, 512], BF, tag="msk")
            ident = const.tile([128, 128], BF, tag="ident")
            xT_sb = acts.tile([128, DC, T], BF, tag="xT")

            nc.sync.dma_start(out=wq_sb, in_=wq.rearrange("(dc p) e -> p dc e", p=128))
            nc.sync.dma_start(out=wk_sb, in_=wk.rearrange("(dc p) e -> p dc e", p=128))
            for dc in range(DC):
                nc.sync.dma_start(
                    out=xT_sb[:, dc, :], in_=xT[dc * 128 : (dc + 1) * 128, :]
                )
            nc.sync.dma_start(
                out=wv_sb, in_=wv.rearrange("(dc p) e -> p dc e", p=128)
            )
            nc.sync.dma_start(
                out=msk_sb, in_=msk.rearrange("r p f -> p r f")
            )
            nc.sync.dma_start(
                out=wo_sb, in_=wo.rearrange("(ec p) o -> p ec o", p=128)
            )
            make_identity(nc, ident)

            QT_sb = acts.tile([128, EC, T], BF, tag="QT")
            KT_sb = acts.tile([128, EC, T], BF, tag="KT")
            V_sb = acts.tile([128, TQ, 8 * 65], BF, tag="V")
            CT_sb = acts.tile([128, EC, T], BF, tag="CT")

            # ones columns only (col 64 of each per-head 65-group) so the V
            # copies below touch disjoint bytes and carry no WAW dep here
            for t7 in range(TQ):
                nc.vector.memset(
                    V_sb[:, t7, :].rearrange("p (h d) -> p h d", d=65)[:, :, 64:65],
                    1.0,
                )

            # ---------- op factories (emitted interleaved, see queue below)
            def qk_proj_ops(cp):
                ops = []
                for dst, w_sb in ((QT_sb, wq_sb), (KT_sb, wk_sb)):
                    for t5 in range(TJ):

                        def op(dst=dst, w_sb=w_sb, cp=cp, t5=t5):
                            ps = psP.tile([128, 512], F32, tag="psP")
                            for dc in range(DC):
                                nc.tensor.matmul(
                                    ps,
                                    lhsT=w_sb[:, dc, cp * 128 : (cp + 1) * 128],
                                    rhs=xT_sb[:, dc, t5 * 512 : (t5 + 1) * 512],
                                    start=(dc == 0),
                                    stop=(dc == DC - 1),
                                )
                            nc.vector.tensor_copy(
                                out=dst[:, cp, t5 * 512 : (t5 + 1) * 512], in_=ps
                            )

                        ops.append(op)
                return ops

            def v_ops():
                ops = []
                for t7 in range(TQ):

                    def op(t7=t7):
                        ps = psP.tile([128, 512], F32, tag="psP")
                        for dc in range(DC):
                            nc.tensor.matmul(
                                ps,
                                lhsT=xT_sb[:, dc, t7 * 128 : (t7 + 1) * 128],
                                rhs=wv_sb[:, dc, :],
                                start=(dc == 0),
                                stop=(dc == DC - 1),
                            )
                        for h8 in range(8):
                            nc.vector.tensor_copy(
                                out=V_sb[:, t7, h8 * 65 : h8 * 65 + 64],
                                in_=ps[:, h8 * 64 : (h8 + 1) * 64],
                            )

                    ops.append(op)
                return ops

            def outproj_ops(j):
                ops = []
                for t7 in range(4 * j, 4 * j + 4):

                    def op(t7=t7):
                        ob = obufp.tile([128, 1024], F32, tag="obuf")
                        for oc in range(2):
                            ps = psP.tile([128, 512], F32, tag="psP")
                            for ec in range(EC):
                                nc.tensor.matmul(
                                    ps,
                                    lhsT=CT_sb[:, ec, t7 * 128 : (t7 + 1) * 128],
                                    rhs=wo_sb[:, ec, oc * 512 : (oc + 1) * 512],
                                    start=(ec == 0),
                                    stop=(ec == EC - 1),
                                )
                            nc.vector.tensor_copy(
                                out=ob[:, oc * 512 : (oc + 1) * 512], in_=ps
                            )
                        nc.sync.dma_start(
                            out=outp[t7 * 128 : (t7 + 1) * 128, :], in_=ob
                        )

                    ops.append(op)
                return ops

            def ctx_ops(cp, j, pts):
                ops = []
                for qr in range(4):
                    qc = 4 * j + qr
                    cn = stage.tile([128, 128], BF, tag="ctxn")
                    for h in range(2):

                        def mm_group(cp=cp, h=h, qr=qr, qc=qc, pts=pts, cn=cn):
                            habs = 2 * cp + h
                            cps = psC.tile([128, 65], F32, tag="psC")
                            for i in range(qc + 1):
                                nc.tensor.matmul(
                                    cps,
                                    lhsT=pts[i][:, h, qr * 128 : (qr + 1) * 128],
                                    rhs=V_sb[:, i, habs * 65 : habs * 65 + 65],
                                    start=(i == 0),
                                    stop=(i == qc),
                                )
                            rc = small.tile([128, 1], F32, tag="recip")
                            nc.vector.reciprocal(rc, cps[:, 64:65])
                            nc.vector.tensor_scalar_mul(
                                out=cn[:, 64 * h : 64 * h + 64],
                                in0=cps[:, 0:64],
                                scalar1=rc,
                            )

                        ops.append(mm_group)

                    def finish(cp=cp, qc=qc, cn=cn):
                        tps = psC.tile([128, 128], BF, tag="psC")
                        nc.tensor.transpose(tps, cn, ident)
                        nc.vector.tensor_copy(
                            out=CT_sb[:, cp, qc * 128 : (qc + 1) * 128], in_=tps
                        )

                    ops.append(finish)
                return ops

            # ---------- pipelined emission ----------
            # S/exp/mask stream is ACT-bound; all other PE work (projections,
            # V, ctx of the previous iteration, out-proj) drains through the
            # two queues between S steps so the PE never starves.
            for op in qk_proj_ops(0):
                op()
            slow = list(v_ops())
            fast = []
            for cp in range(EC):
                if cp < EC - 1:
                    slow.extend(qk_proj_ops(cp + 1))
                steps_left = 40
                for j in range(TJ):
                    nk = 4 * j + 4
                    ds = max(1, nk // 2)
                    fper = (len(fast) + ds - 1) // ds
                    pts = []
                    for i in range(nk):
                        pt = ppool.tile([128, 2, 512], BF, tag="P")
                        sh = psS.tile([128, 2, 512], F32, tag="psS")
                        for h in range(2):
                            lo = 64 * h
                            nc.tensor.matmul(
                                sh[:, h, :],
                                lhsT=KT_sb[lo : lo + 64, cp, i * 128 : (i + 1) * 128],
                                rhs=QT_sb[lo : lo + 64, cp, j * 512 : (j + 1) * 512],
                                start=True,
                                stop=True,
                            )
                        nc.scalar.activation(
                            out=pt,
                            in_=sh,
                            func=mybir.ActivationFunctionType.Exp,
                            scale=0.125,
                        )
                        r = i - 4 * j
                        if r >= 0:  # diagonal tile: causal mask (f >= p+128r)
                            for h in range(2):
                                nc.vector.tensor_mul(
                                    pt[:, h, :], pt[:, h, :], msk_sb[:, r, :]
                                )
                        pts.append(pt)
                        for _ in range(fper):
                            if fast:
                                fast.pop(0)()
                        spr = (len(slow) + steps_left - 1) // steps_left
                        for _ in range(spr):
                            if slow:
                                slow.pop(0)()
                        steps_left -= 1
                    while fast:
                        fast.pop(0)()
                    fast = ctx_ops(cp, j, pts)
                    if cp == EC - 1:
                        # out-proj for these q rows AFTER their ctx writes in
                        # the same FIFO: queued ops can only depend on already
                        # emitted producers
                        fast.extend(outproj_ops(j))
            while fast:
                fast.pop(0)()
            while slow:
                slow.pop(0)()
    nc.compile()
    return nc


def _get_nc():
    if "nc" not in _CACHE:
        _CACHE["nc"] = _build()
    return _CACHE["nc"]


def _ensure_ntff_hook():
    """Install the axon NTFF profiling hook if the image's antenv lacks it."""
    import sys
    import types

    try:
        import antenv.axon_hooks  # noqa: F401

        return
    except ImportError:
        pass
    try:
        import antenv

        mod = types.ModuleType("antenv.axon_hooks")
        holder = {"hook": None}
        mod.set_axon_ntff_profile_hook = lambda h: holder.__setitem__("hook", h)
        mod.get_axon_ntff_profile_hook = lambda: holder["hook"]
        sys.modules["antenv.axon_hooks"] = mod
        antenv.axon_hooks = mod
        from trn_agent_boot.trn_boot import _ntff_profile_via_ctypes

        so = "/opt/axon/libaxon_pjrt.so"
        if os.path.exists(so):
            mod.set_axon_ntff_profile_hook(_ntff_profile_via_ctypes(so))
    except Exception:
        pass


def _masks():
    p = np.arange(128)[:, None]
    f = np.arange(512)[None, :]
    return np.stack(
        [(f >= p + 128 * r).astype(np.float32) for r in range(4)]
    ).astype(BF16)


def kernel(inputs, Wq, Wk, Wv, Wo, bo):
    inputs = np.asarray(inputs, dtype=np.float32)
    Wq = np.asarray(Wq, dtype=np.float32)
    Wk = np.asarray(Wk, dtype=np.float32)
    Wv = np.asarray(Wv, dtype=np.float32)
    Wo = np.asarray(Wo, dtype=np.float32)
    bo = np.asarray(bo, dtype=np.float32)

    nc = _get_nc()
    msk = _masks()
    wqs = [np.ascontiguousarray(Wq[:, g * E : (g + 1) * E]).astype(BF16) for g in range(2)]
    wks = [np.ascontiguousarray(Wk[:, g * E : (g + 1) * E]).astype(BF16) for g in range(2)]
    wvs = [np.ascontiguousarray(Wv[:, g * E : (g + 1) * E]).astype(BF16) for g in range(2)]
    wos = [np.ascontiguousarray(Wo[g * E : (g + 1) * E, :]).astype(BF16) for g in range(2)]
    xTs = [np.ascontiguousarray(inputs[b].T).astype(BF16) for b in range(B)]

    in_maps = []
    for c in range(8):
        b, g = divmod(c, 2)
        in_maps.append(
            {
                "xT": xTs[b],
                "wq": wqs[g],
                "wk": wks[g],
                "wv": wvs[g],
                "wo": wos[g],
                "msk": msk,
            }
        )

    trace = os.environ.get("KERNEL_TRACE", "0") == "1"
    if trace:
        _ensure_ntff_hook()
    tcores = None
    if os.environ.get("KERNEL_TRACE_ALL", "0") == "1":
        tcores = list(range(8))
    res = run_bass_kernel_spmd(
        nc, in_maps, core_ids=list(range(8)), trace=trace, trace_cores=tcores
    )
    LAST["exec_ns"] = res.exec_time_ns
    LAST["trace"] = res.instructions_and_trace
    LAST["profile_json"] = res.profile_json

    out = np.empty((B, T, D), np.float32)
    for b in range(B):
        out[b] = res.results[2 * b]["out"] + res.results[2 * b + 1]["out"] + bo[None, :]
    return out


# revision 23
# speedup vs baseline: 1.2206x; 1.0208x over previous
"""Causal MHA (B=4, T=2048, D=1024, H=16) on 8 trn2 cores.

Sharding: core c = (batch b = c//2, head-group g = c%2). Each core computes
QKV projections for its 8 heads, causal attention, and the row-parallel
out-proj partial product. Host sums the two partials per batch + bias.

On-device layout (per core):
  xT   [1024, 2048]  X^T (d on partitions)           bf16
  QT/KT [512, 2048]  Q^T/K^T (e=head*64+d rows)      bf16
  V_pad [2048, 520]  V natural + ones col per head   bf16
  scores S^T tiles [128 k, 2x512 q] (2 heads/psum), exp on ScalarE,
  ctx = P^T-stationary matmul -> [128 q, 65] (col 64 = softmax denom),
  normalize per-partition, PE-transpose -> ctx^T, out-proj partial.
"""

import os

import numpy as np
import ml_dtypes

import concourse.bass as bass
import concourse.bacc as bacc
import concourse.tile as tile
from concourse import mybir
from concourse.bass_utils import run_bass_kernel_spmd
from concourse.masks import make_identity

BF16 = ml_dtypes.bfloat16

B, T, D = 4, 2048, 1024
H, HD = 16, 64
E = 512          # per-core projection width (8 heads * 64)
DC = D // 128    # 8 contraction chunks
EC = E // 128    # 4 e chunks (head pairs)
TJ = T // 512    # 4 q-chunks of 512
TQ = T // 128    # 16 t-chunks of 128

F32 = mybir.dt.float32
BF = mybir.dt.bfloat16

LAST = {}
_CACHE = {}


def _build():
    nc = bacc.Bacc("TRN2")
    xT = nc.dram_tensor("xT", [D, T], BF, kind="ExternalInput")
    wq = nc.dram_tensor("wq", [D, E], BF, kind="ExternalInput")
    wk = nc.dram_tensor("wk", [D, E], BF, kind="ExternalInput")
    wv = nc.dram_tensor("wv", [D, E], BF, kind="ExternalInput")
    wo = nc.dram_tensor("wo", [E, D], BF, kind="ExternalInput")
    msk = nc.dram_tensor("msk", [4, 128, 512], BF, kind="ExternalInput")
    outp = nc.dram_tensor("out", [T, D], F32, kind="ExternalOutput")

    with tile.TileContext(nc) as tc:
        with (
            tc.tile_pool(name="const", bufs=1) as const,
            tc.tile_pool(name="acts", bufs=1) as acts,
            tc.tile_pool(name="ppool", bufs=24) as ppool,
            tc.tile_pool(name="small", bufs=6) as small,
            tc.tile_pool(name="stage", bufs=6) as stage,
            tc.tile_pool(name="obuf", bufs=2) as obufp,
            tc.tile_pool(name="psS", bufs=2, space="PSUM") as psS,
            tc.tile_pool(name="psP", bufs=2, space="PSUM") as psP,
            tc.tile_pool(name="psC", bufs=2, space="PSUM") as psC,
        ):
            # ---------- constants; DMA order follows the critical path:
            # wq/wk + xT gate the first projections, wv/wo/msk can trail
            wq_sb = const.tile([128, DC, E], BF, tag="wq")
            wk_sb = const.tile([128, DC, E], BF, tag="wk")
            wv_sb = const.tile([128, DC, E], BF, tag="wv")
            wo_sb = const.tile([128, EC, D], BF, tag="wo")
            msk_sb = const.tile([128, 4, 512], BF, tag="msk")
            ident = const.tile([128, 128], BF, tag="ident")
            xT_sb = acts.tile([128, DC, T], BF, tag="xT")

            nc.sync.dma_start(out=wq_sb, in_=wq.rearrange("(dc p) e -> p dc e", p=128))
            nc.sync.dma_start(out=wk_sb, in_=wk.rearrange("(dc p) e -> p dc e", p=128))
            for th in range(2):
                for dc in range(DC):
                    nc.sync.dma_start(
                        out=xT_sb[:, dc, th * 1024 : (th + 1) * 1024],
                        in_=xT[dc * 128 : (dc + 1) * 128, th * 1024 : (th + 1) * 1024],
                    )
            nc.sync.dma_start(
                out=wv_sb, in_=wv.rearrange("(dc p) e -> p dc e", p=128)
            )
            nc.sync.dma_start(
                out=msk_sb, in_=msk.rearrange("r p f -> p r f")
            )
            nc.sync.dma_start(
                out=wo_sb, in_=wo.rearrange("(ec p) o -> p ec o", p=128)
            )
            make_identity(nc, ident)

            QT_sb = acts.tile([128, EC, T], BF, tag="QT")
            KT_sb = acts.tile([128, EC, T], BF, tag="KT")
            V_sb = acts.tile([128, TQ, 8 * 65], BF, tag="V")
            CT_sb = acts.tile([128, EC, T], BF, tag="CT")

            # ones columns only (col 64 of each per-head 65-group) so the V
            # copies below touch disjoint bytes and carry no WAW dep here
            for t7 in range(TQ):
                nc.vector.memset(
                    V_sb[:, t7, :].rearrange("p (h d) -> p h d", d=65)[:, :, 64:65],
                    1.0,
                )

            # ---------- op factories (emitted interleaved, see queue below)
            def qk_proj_ops(cp):
                ops = []
                for dst, w_sb in ((QT_sb, wq_sb), (KT_sb, wk_sb)):
                    for t5 in range(TJ):

                        def op(dst=dst, w_sb=w_sb, cp=cp, t5=t5):
                            ps = psP.tile([128, 512], F32, tag="psP")
                            for dc in range(DC):
                                nc.tensor.matmul(
                                    ps,
                                    lhsT=w_sb[:, dc, cp * 128 : (cp + 1) * 128],
                                    rhs=xT_sb[:, dc, t5 * 512 : (t5 + 1) * 512],
                                    start=(dc == 0),
                                    stop=(dc == DC - 1),
                                )
                            nc.vector.tensor_copy(
                                out=dst[:, cp, t5 * 512 : (t5 + 1) * 512], in_=ps
                            )

                        ops.append(op)
                return ops

            def v_ops():
                ops = []
                for t7 in range(TQ):

                    def op(t7=t7):
                        ps = psP.tile([128, 512], F32, tag="psP")
                        for dc in range(DC):
                            nc.tensor.matmul(
                                ps,
                                lhsT=xT_sb[:, dc, t7 * 128 : (t7 + 1) * 128],
                                rhs=wv_sb[:, dc, :],
                                start=(dc == 0),
                                stop=(dc == DC - 1),
                            )
                        for h8 in range(8):
                            nc.vector.tensor_copy(
                                out=V_sb[:, t7, h8 * 65 : h8 * 65 + 64],
                                in_=ps[:, h8 * 64 : (h8 + 1) * 64],
                            )

                    ops.append(op)
                return ops

            def outproj_ops(j):
                ops = []
                for t7 in range(4 * j, 4 * j + 4):

                    def op(t7=t7):
                        ob = obufp.tile([128, 1024], F32, tag="obuf")
                        for oc in range(2):
                            ps = psP.tile([128, 512], F32, tag="psP")
                            for ec in range(EC):
                                nc.tensor.matmul(
                                    ps,
                                    lhsT=CT_sb[:, ec, t7 * 128 : (t7 + 1) * 128],
                                    rhs=wo_sb[:, ec, oc * 512 : (oc + 1) * 512],
                                    start=(ec == 0),
                                    stop=(ec == EC - 1),
                                )
                            nc.vector.tensor_copy(
                                out=ob[:, oc * 512 : (oc + 1) * 512], in_=ps
                            )
                        nc.sync.dma_start(
                            out=outp[t7 * 128 : (t7 + 1) * 128, :], in_=ob
                        )

                    ops.append(op)
                return ops

            def ctx_ops(cp, j, pts):
                ops = []
                for qr in range(4):
                    qc = 4 * j + qr
                    cn = stage.tile([128, 128], BF, tag="ctxn")
                    for h in range(2):

                        def mm_group(cp=cp, h=h, qr=qr, qc=qc, pts=pts, cn=cn):
                            habs = 2 * cp + h
                            cps = psC.tile([128, 65], F32, tag="psC")
                            for i in range(qc + 1):
                                nc.tensor.matmul(
                                    cps,
                                    lhsT=pts[i][:, h, qr * 128 : (qr + 1) * 128],
                                    rhs=V_sb[:, i, habs * 65 : habs * 65 + 65],
                                    start=(i == 0),
                                    stop=(i == qc),
                                )
                            rc = small.tile([128, 1], F32, tag="recip")
                            nc.vector.reciprocal(rc, cps[:, 64:65])
                            nc.vector.tensor_scalar_mul(
                                out=cn[:, 64 * h : 64 * h + 64],
                                in0=cps[:, 0:64],
                                scalar1=rc,
                            )

                        ops.append(mm_group)

                    def finish(cp=cp, qc=qc, cn=cn):
                        tps = psC.tile([128, 128], BF, tag="psC")
                        nc.tensor.transpose(tps, cn, ident)
                        nc.vector.tensor_copy(
                            out=CT_sb[:, cp, qc * 128 : (qc + 1) * 128], in_=tps
                        )

                    ops.append(finish)
                return ops

            # ---------- pipelined emission ----------
            # S/exp/mask stream is ACT-bound; all other PE work (projections,
            # V, ctx of the previous iteration, out-proj) drains through the
            # two queues between S steps so the PE never starves.
            for op in qk_proj_ops(0):
                op()
            slow = list(v_ops())
            fast = []
            for cp in range(EC):
                if cp < EC - 1:
                    slow.extend(qk_proj_ops(cp + 1))
                steps_left = 40
                for j in range(TJ):
                    nk = 4 * j + 4
                    ds = max(1, nk // 2)
                    fper = (len(fast) + ds - 1) // ds
                    pts = []
                    for i in range(nk):
                        pt = ppool.tile([128, 2, 512], BF, tag="P")
                        sh = psS.tile([128, 2, 512], F32, tag="psS")
                        for h in range(2):
                            lo = 64 * h
                            nc.tensor.matmul(
                                sh[:, h, :],
                                lhsT=KT_sb[lo : lo + 64, cp, i * 128 : (i + 1) * 128],
                                rhs=QT_sb[lo : lo + 64, cp, j * 512 : (j + 1) * 512],
                                start=True,
                                stop=True,
                            )
                        nc.scalar.activation(
                            out=pt,
                            in_=sh,
                            func=mybir.ActivationFunctionType.Exp,
                            scale=0.125,
                        )
                        r = i - 4 * j
                        if r >= 0:  # diagonal tile: causal mask (f >= p+128r)
                            # keep where f - p - 128r >= 0, else 0 (GpSimd is
                            # otherwise idle; saves DVE time)
                            for h in range(2):
                                nc.gpsimd.affine_select(
                                    out=pt[:, h, :],
                                    in_=pt[:, h, :],
                                    compare_op=mybir.AluOpType.is_ge,
                                    fill=0.0,
                                    base=-128 * r,
                                    pattern=[[1, 512]],
                                    channel_multiplier=-1,
                                )
                        pts.append(pt)
                        for _ in range(fper):
                            if fast:
                                fast.pop(0)()
                        spr = (len(slow) + steps_left - 1) // steps_left
                        for _ in range(spr):
                            if slow:
                                slow.pop(0)()
                        steps_left -= 1
                    while fast:
                        fast.pop(0)()
                    fast = ctx_ops(cp, j, pts)
                    if cp == EC - 1:
                        # out-proj for these q rows AFTER their ctx writes in
                        # the same FIFO: queued ops can only depend on already
                        # emitted producers
                        fast.extend(outproj_ops(j))
            while fast:
                fast.pop(0)()
            while slow:
                slow.pop(0)()
    nc.compile()
    return nc


def _get_nc():
    if "nc" not in _CACHE:
        _CACHE["nc"] = _build()
    return _CACHE["nc"]


def _ensure_ntff_hook():
    """Install the axon NTFF profiling hook if the image's antenv lacks it."""
    import sys
    import types

    try:
        import antenv.axon_hooks  # noqa: F401

        return
    except ImportError:
        pass
    try:
        import antenv

        mod = types.ModuleType("antenv.axon_hooks")
        holder = {"hook": None}
        mod.set_axon_ntff_profile_hook = lambda h: holder.__setitem__("hook", h)
        mod.get_axon_ntff_profile_hook = lambda: holder["hook"]
        sys.modules["antenv.axon_hooks"] = mod
        antenv.axon_hooks = mod
        from trn_agent_boot.trn_boot import _ntff_profile_via_ctypes

        so = "/opt/axon/libaxon_pjrt.so"
        if os.path.exists(so):
            mod.set_axon_ntff_profile_hook(_ntff_profile_via_ctypes(so))
    except Exception:
        pass


def _masks():
    p = np.arange(128)[:, None]
    f = np.arange(512)[None, :]
    return np.stack(
        [(f >= p + 128 * r).astype(np.float32) for r in range(4)]
    ).astype(BF16)


def kernel(inputs, Wq, Wk, Wv, Wo, bo):
    inputs = np.asarray(inputs, dtype=np.float32)
    Wq = np.asarray(Wq, dtype=np.float32)
    Wk = np.asarray(Wk, dtype=np.float32)
    Wv = np.asarray(Wv, dtype=np.float32)
    Wo = np.asarray(Wo, dtype=np.float32)
    bo = np.asarray(bo, dtype=np.float32)

    nc = _get_nc()
    msk = _masks()
    wqs = [np.ascontiguousarray(Wq[:, g * E : (g + 1) * E]).astype(BF16) for g in range(2)]
    wks = [np.ascontiguousarray(Wk[:, g * E : (g + 1) * E]).astype(BF16) for g in range(2)]
    wvs = [np.ascontiguousarray(Wv[:, g * E : (g + 1) * E]).astype(BF16) for g in range(2)]
    wos = [np.ascontiguousarray(Wo[g * E : (g + 1) * E, :]).astype(BF16) for g in range(2)]
    xTs = [np.ascontiguousarray(inputs[b].T).astype(BF16) for b in range(B)]

    in_maps = []
    for c in range(8):
        b, g = divmod(c, 2)
        in_maps.append(
            {
                "xT": xTs[b],
                "wq": wqs[g],
                "wk": wks[g],
                "wv": wvs[g],
                "wo": wos[g],
                "msk": msk,
            }
        )

    trace = os.environ.get("KERNEL_TRACE", "0") == "1"
    if trace:
        _ensure_ntff_hook()
    tcores = None
    if os.environ.get("KERNEL_TRACE_ALL", "0") == "1":
        tcores = list(range(8))
    res = run_bass_kernel_spmd(
        nc, in_maps, core_ids=list(range(8)), trace=trace, trace_cores=tcores
    )
    LAST["exec_ns"] = res.exec_time_ns
    LAST["trace"] = res.instructions_and_trace
    LAST["profile_json"] = res.profile_json

    out = np.empty((B, T, D), np.float32)
    for b in range(B):
        out[b] = res.results[2 * b]["out"] + res.results[2 * b + 1]["out"] + bo[None, :]
    return out
